# revision 6
# baseline (speedup 1.0000x reference)
"""Trainium2 Bass kernel for a cross-modal transformer block (CrossModalAttention).

Reference computation (per batch b):
  q = x1 @ Wq + bq ; k = x2 @ Wk + bk ; v = x2 @ Wv + bv    (H=16 heads, HD=64)
  attn = softmax(q k^T / sqrt(HD)) ; out = attn @ v
  h  = LN(x1 + out @ Wo + bo; g1, b1)
  y  = LN(h + gelu(h @ Wf1 + bf1) @ Wf2 + bf2; g2, b2)

Shapes: B=4, S=2048, D=1024, DFF=4096, fp32 in/out.

Sharding (8 cores, no collectives): core c handles batch c//2, query rows
(c%2)*1024 .. +1024. Each core recomputes K/V for its batch (only the K/V
projections are duplicated 2x; everything else is perfectly data-parallel).

On-chip strategy (per core, TQ=1024 queries, TK=2048 keys):
  - activations kept feature-major ("transposed", [D on partitions, tokens
    free]) through QKV and attention; bf16 matmul inputs, fp32 PSUM.
  - scores computed directly as S^T tiles [tk, tq] via matmul(lhsT=KT chunk,
    rhs=QT chunk); the two heads living in one 128-partition feature chunk
    run concurrently in the PE array (row groups 0-1 vs 2-3).
  - softmax without max subtraction (scores ~ N(0,1); exp is safe in fp32):
    exp on ScalarE (scale=1/8 folded in), PSUM -> SBUF bf16.
  - P@V via lhsT=[V_head | ones] ([tk,65]): row 64 of the PSUM output
    accumulates the softmax denominators for free; a K=1 ones-matmul
    broadcasts 1/denominator across partitions for the normalization.
  - out-projection (lhsT=OT chunks) lands token-major; LayerNorms run
    token-major with bn_stats/bn_aggr; h1 is transposed back (PE transpose)
    for the FFN; the FFN runs in DFF slices (SBUF budget), partial FFN2
    results accumulate into the h1 buffer (h1 itself is only needed for the
    second residual), and the final LN writes fp32 output directly.
"""

import numpy as np
import ml_dtypes

import concourse.bass as bass
import concourse.tile as tile
import concourse.mybir as mybir
from concourse import bass_utils
from concourse.masks import make_identity

F32 = mybir.dt.float32
BF16 = mybir.dt.bfloat16
AF = mybir.ActivationFunctionType
ADD = mybir.AluOpType.add
BF = ml_dtypes.bfloat16

D = 1024
H = 16
HD = 64
DFF = 4096
EPS = 1e-5
KO = D // 128        # feature chunks of D
DFFC = DFF // 128    # feature chunks of DFF
NDH = 2              # FFN processed in this many DFF slices (SBUF budget)
P = 128

_nc_cache = {}

# ---------------------------------------------------------------------------
# Workaround: this container's walrus build rejects >1 sync wait per
# instruction ("Too many sync wait commands"). Hoist all but one wait of each
# instruction onto NoOps inserted before it on the same engine stream.
_ws_ctr = [0]


def _split_multi_waits(nc):
    for fn in nc.m.functions:
        for bb in fn.blocks:
            out = []
            changed = False
            for inst in bb.instructions:
                si = inst.sync_info
                if si is not None and si.on_wait and len(si.on_wait) > 1:
                    waits = list(si.on_wait)
                    for w in waits[:-1]:
                        _ws_ctr[0] += 1
                        nop = mybir.InstNoOp(
                            name=f"waitsplit-{_ws_ctr[0]}", ins=[], outs=[]
                        )
                        nop.engine = inst.engine
                        nop.sync_info = mybir.SyncInfo(on_wait=[w], on_update=[])
                        out.append(nop)
                    si.on_wait = [waits[-1]]
                    changed = True
                out.append(inst)
            if changed:
                bb.instructions[:] = out
    return nc


def _chunks(total, size):
    return [(s, min(size, total - s)) for s in range(0, total, size)]


# ---------------------------------------------------------------------------
def build_nc(TQ=1024, TK=2048):
    """Build the single-core Bass program (same program runs SPMD on 8 cores)."""
    TQT = TQ // P       # query 128-tiles
    TKT = TK // P       # key 128-tiles
    DH = DFFC // NDH    # dff chunks per slice
    nc = bass.Bass(trn_type="TRN2")

    # DRAM inputs (host pre-layouts everything into SBUF-friendly shapes)
    x1t = nc.dram_tensor("x1t", [P, KO, TQ], BF16, kind="ExternalInput")
    x1b = nc.dram_tensor("x1b", [TQ, D], F32, kind="ExternalInput")  # x1 + bo
    x2t = nc.dram_tensor("x2t", [P, KO, TK], BF16, kind="ExternalInput")
    wq = nc.dram_tensor("wq", [P, KO, D], BF16, kind="ExternalInput")
    wk = nc.dram_tensor("wk", [P, KO, D], BF16, kind="ExternalInput")
    wv = nc.dram_tensor("wv", [P, KO, D], BF16, kind="ExternalInput")
    wo = nc.dram_tensor("wo", [P, KO, D], BF16, kind="ExternalInput")
    wf1 = nc.dram_tensor("wf1", [DFFC, P, KO, P], BF16, kind="ExternalInput")
    wf2 = nc.dram_tensor("wf2", [DFFC, P, D], BF16, kind="ExternalInput")
    bq = nc.dram_tensor("bq", [P, KO], F32, kind="ExternalInput")
    bk = nc.dram_tensor("bk", [P, KO], F32, kind="ExternalInput")
    bvr = nc.dram_tensor("bvr", [1, D], F32, kind="ExternalInput")
    bf1 = nc.dram_tensor("bf1", [P, DFFC], F32, kind="ExternalInput")
    bf2r = nc.dram_tensor("bf2r", [1, D], F32, kind="ExternalInput")
    g1r = nc.dram_tensor("g1r", [1, D], F32, kind="ExternalInput")
    b1r = nc.dram_tensor("b1r", [1, D], F32, kind="ExternalInput")
    g2r = nc.dram_tensor("g2r", [1, D], F32, kind="ExternalInput")
    b2r = nc.dram_tensor("b2r", [1, D], F32, kind="ExternalInput")
    y = nc.dram_tensor("y", [TQ, D], F32, kind="ExternalOutput")

    def bcast_row(row):  # [1, D] dram -> broadcast AP over 128 partitions
        return bass.AP(tensor=row.tensor, offset=row.offset,
                       ap=[[0, P], row.ap[1]])

    with tile.TileContext(nc) as tc:
        with tc.tile_pool(name="consts", bufs=1) as consts:
            ident = consts.tile([P, P], F32)
            make_identity(nc, ident)
            ones_f = consts.tile([P, HD], F32)
            nc.vector.memset(ones_f, 1.0)
            eps_sb = consts.tile([P, 1], F32)
            nc.vector.memset(eps_sb, EPS)
            bq_sb = consts.tile([P, KO], F32)
            nc.sync.dma_start(bq_sb[:], bq[:])
            bk_sb = consts.tile([P, KO], F32)
            nc.sync.dma_start(bk_sb[:], bk[:])
            bf1_sb = consts.tile([P, DFFC], F32)
            nc.sync.dma_start(bf1_sb[:], bf1[:])

            def load_bcast(row):
                t = consts.tile([P, D], F32, name=f"bc_{row.name}")
                nc.sync.dma_start(t[:], bcast_row(row[:]))
                return t

            bv_b = load_bcast(bvr)
            bf2_b = load_bcast(bf2r)
            g1_b = load_bcast(g1r)
            b1_b = load_bcast(b1r)
            g2_b = load_bcast(g2r)
            b2_b = load_bcast(b2r)

            self_scope = tc.tile_pool(name="oacts", bufs=1)
            with self_scope as oacts:
                ot_sb = oacts.tile([P, KO, TQ], BF16)

                with tc.tile_pool(name="attn_acts", bufs=1) as attn_acts:
                    qt_sb = attn_acts.tile([P, KO, TQ], BF16)
                    kt_sb = attn_acts.tile([P, KO, TK], BF16)
                    v_sb = attn_acts.tile([P, TKT, H, HD + 1], BF16)

                    # --------- QKV projections (feature-major) -------------
                    with (
                        tc.tile_pool(name="qkv_w", bufs=2) as wpool,
                        tc.tile_pool(name="qkv_x", bufs=1) as xpool,
                        tc.tile_pool(name="qkv_ps", bufs=2,
                                     space="PSUM") as qkv_ps,
                    ):
                        x1t_sb = xpool.tile([P, KO, TK], BF16,
                                            tag="x", name="x1t_sb")[:, :, :TQ]
                        nc.sync.dma_start(x1t_sb[:], x1t[:])
                        wq_sb = wpool.tile([P, KO, D], BF16, tag="w",
                                           name="wq_sb")
                        nc.sync.dma_start(wq_sb[:], wq[:])
                        for n in range(KO):
                            ps = qkv_ps.tile([P, 1024], F32, tag="ps")
                            for kc in range(KO):
                                for fs, fl in _chunks(TQ, 512):
                                    nc.tensor.matmul(
                                        ps[:, fs:fs + fl],
                                        wq_sb[:, kc, n * P:(n + 1) * P],
                                        x1t_sb[:, kc, fs:fs + fl],
                                        start=(kc == 0), stop=(kc == KO - 1),
                                    )
                            nc.vector.tensor_scalar_add(
                                qt_sb[:, n, :], ps[:, :TQ], bq_sb[:, n:n + 1])

                        x2t_sb = xpool.tile([P, KO, TK], BF16, tag="x",
                                            name="x2t_sb")
                        nc.sync.dma_start(x2t_sb[:], x2t[:])
                        wk_sb = wpool.tile([P, KO, D], BF16, tag="w",
                                           name="wk_sb")
                        nc.sync.dma_start(wk_sb[:], wk[:])
                        for n in range(KO):
                            for hs, hl in _chunks(TK, 1024):
                                ps = qkv_ps.tile([P, 1024], F32, tag="ps")
                                for kc in range(KO):
                                    for fs, fl in _chunks(hl, 512):
                                        nc.tensor.matmul(
                                            ps[:, fs:fs + fl],
                                            wk_sb[:, kc, n * P:(n + 1) * P],
                                            x2t_sb[:, kc,
                                                   hs + fs:hs + fs + fl],
                                            start=(kc == 0),
                                            stop=(kc == KO - 1),
                                        )
                                nc.vector.tensor_scalar_add(
                                    kt_sb[:, n, hs:hs + hl], ps[:, :hl],
                                    bk_sb[:, n:n + 1])

                        # V token-major, augmented ones column per head
                        nc.vector.memset(v_sb[:], 1.0)
                        wv_sb = wpool.tile([P, KO, D], BF16, tag="w",
                                           name="wv_sb")
                        nc.sync.dma_start(wv_sb[:], wv[:])
                        for t in range(TKT):
                            ps = qkv_ps.tile([P, 1024], F32, tag="ps")
                            for kc in range(KO):
                                for fs, fl in _chunks(D, 512):
                                    nc.tensor.matmul(
                                        ps[:, fs:fs + fl],
                                        x2t_sb[:, kc, t * P:(t + 1) * P],
                                        wv_sb[:, kc, fs:fs + fl],
                                        start=(kc == 0), stop=(kc == KO - 1),
                                    )
                            nc.vector.tensor_tensor(
                                v_sb[:, t, :, 0:HD],
                                ps[:, :D].rearrange("p (h c) -> p h c", c=HD),
                                bv_b[:].rearrange("p (h c) -> p h c", c=HD),
                                ADD,
                            )

                    # --------- attention -----------------------------------
                    with (
                        tc.tile_pool(name="att_s", bufs=2, space="PSUM") as s_ps,
                        tc.tile_pool(name="att_pv", bufs=3,
                                     space="PSUM") as pv_ps,
                        tc.tile_pool(name="att_nm", bufs=1,
                                     space="PSUM") as nm_ps,
                        tc.tile_pool(name="att_pt", bufs=4) as pt_pool,
                        tc.tile_pool(name="att_sm", bufs=3) as sm_pool,
                    ):
                        for ko in range(KO):      # head pair (2ko, 2ko+1)
                            for qs, ql in _chunks(TQ, 512):
                                pv = [pv_ps.tile([HD + 1, 512], F32, tag="pv",
                                                 name=f"pv{e}")[:, :ql]
                                      for e in range(2)]
                                for t in range(TKT):
                                    sc = s_ps.tile([P, 2, 512], F32, tag="sc")
                                    for e in range(2):
                                        nc.tensor.matmul(
                                            sc[:, e, :ql],
                                            kt_sb[64 * e:64 * e + 64, ko,
                                                  t * P:(t + 1) * P],
                                            qt_sb[64 * e:64 * e + 64, ko,
                                                  qs:qs + ql],
                                        )
                                    pt = pt_pool.tile([P, 2, 512], BF16,
                                                      tag="pt")
                                    nc.scalar.activation(
                                        pt[:, :, :ql], sc[:, :, :ql], AF.Exp,
                                        scale=1.0 / float(np.sqrt(HD)))
                                    for e in range(2):
                                        nc.tensor.matmul(
                                            pv[e][:],
                                            v_sb[:, t, 2 * ko + e, :],
                                            pt[:, e, :ql],
                                            start=(t == 0),
                                            stop=(t == TKT - 1),
                                        )
                                for e in range(2):
                                    rec = sm_pool.tile([P, 512], F32,
                                                       tag="rec")
                                    nc.vector.reciprocal(
                                        rec[64:65, :ql], pv[e][HD:HD + 1, :])
                                    nm = nm_ps.tile([HD, 512], F32, tag="nm")
                                    nc.tensor.matmul(
                                        nm[:, :ql], ones_f[64:65, 0:HD],
                                        rec[64:65, :ql])
                                    nms = sm_pool.tile([HD, 512], F32,
                                                       tag="nms")
                                    nc.vector.tensor_copy(
                                        nms[:, :ql], nm[:, :ql])
                                    nc.vector.tensor_mul(
                                        ot_sb[64 * e:64 * e + 64, ko,
                                              qs:qs + ql],
                                        pv[e][0:HD, :], nms[:, :ql])

                # attn_acts closed: qt/kt/v freed; FFN buffers reuse the space
                with tc.tile_pool(name="ffn_acts", bufs=1) as ffn_acts:
                    h1_sb = ffn_acts.tile([P, TQT, D], F32)
                    h1t_sb = ffn_acts.tile([P, KO, TQ], BF16)

                    # --------- out-proj + LN1 + transpose ------------------
                    with (
                        tc.tile_pool(name="oproj_w", bufs=1) as owpool,
                        tc.tile_pool(name="oproj_ps", bufs=2,
                                     space="PSUM") as o_ps,
                        tc.tile_pool(name="tr_ps", bufs=2,
                                     space="PSUM") as t_ps,
                        tc.tile_pool(name="ln1", bufs=3) as ln1_pool,
                    ):
                        wo_sb = owpool.tile([P, KO, D], BF16)
                        nc.sync.dma_start(wo_sb[:], wo[:])
                        for tq in range(TQT):
                            ps = o_ps.tile([P, D], F32, tag="po")
                            for kc in range(KO):
                                for fs, fl in _chunks(D, 512):
                                    nc.tensor.matmul(
                                        ps[:, fs:fs + fl],
                                        ot_sb[:, kc, tq * P:(tq + 1) * P],
                                        wo_sb[:, kc, fs:fs + fl],
                                        start=(kc == 0), stop=(kc == KO - 1),
                                    )
                            x1bt = ln1_pool.tile([P, D], F32, tag="x1b")
                            nc.sync.dma_start(
                                x1bt[:], x1b[tq * P:(tq + 1) * P, :])
                            resid = ln1_pool.tile([P, D], F32, tag="resid")
                            nc.vector.tensor_add(resid[:], ps[:], x1bt[:])
                            st6 = ln1_pool.tile([P, 2, 6], F32, tag="st6")
                            nc.vector.bn_stats(st6[:, 0, :], resid[:, 0:512])
                            nc.vector.bn_stats(st6[:, 1, :],
                                               resid[:, 512:1024])
                            mv = ln1_pool.tile([P, 2], F32, tag="mv")
                            nc.vector.bn_aggr(mv[:], st6[:])
                            rstd = ln1_pool.tile([P, 1], F32, tag="rstd")
                            nc.scalar.activation(rstd[:], mv[:, 1:2], AF.Sqrt,
                                                 bias=eps_sb[:, 0:1])
                            nc.vector.reciprocal(rstd[:], rstd[:])
                            h1_tile = h1_sb[:, tq, :]
                            nc.vector.tensor_scalar(
                                h1_tile, resid[:], mv[:, 0:1], rstd[:],
                                mybir.AluOpType.subtract,
                                mybir.AluOpType.mult)
                            nc.vector.tensor_mul(h1_tile, h1_tile, g1_b[:])
                            nc.vector.tensor_add(h1_tile, h1_tile, b1_b[:])
                            for kc in range(KO):
                                tp = t_ps.tile([P, P], F32, tag="tp")
                                nc.tensor.transpose(
                                    tp[:], h1_tile[:, kc * P:(kc + 1) * P],
                                    ident[:])
                                nc.vector.tensor_copy(
                                    h1t_sb[:, kc, tq * P:(tq + 1) * P],
                                    tp[:])

                    # --------- FFN (DFF sliced; G accumulates into h1) -----
                    with (
                        tc.tile_pool(name="ffn_big", bufs=1) as big_pool,
                        tc.tile_pool(name="f1w", bufs=4) as f1w_pool,
                        tc.tile_pool(name="f1_ps", bufs=2,
                                     space="PSUM") as f1_ps,
                        tc.tile_pool(name="f2_ps", bufs=2,
                                     space="PSUM") as f2_ps,
                        tc.tile_pool(name="ln2", bufs=3) as ln2_pool,
                    ):
                        for dh in range(NDH):
                            ft_sb = big_pool.tile([P, DH, TQ], BF16,
                                                  tag="ft", name="ft")
                            wf2_sb = big_pool.tile([P, DH, D], BF16,
                                                   tag="wf2", name="wf2h")
                            for ci in range(DH):
                                c = dh * DH + ci
                                nc.sync.dma_start(wf2_sb[:, ci, :], wf2[c])
                                w1t = f1w_pool.tile([P, KO, P], BF16,
                                                    tag="w1")
                                nc.sync.dma_start(w1t[:], wf1[c])
                                ps = f1_ps.tile([P, 1024], F32, tag="pf")
                                for kc in range(KO):
                                    for fs, fl in _chunks(TQ, 512):
                                        nc.tensor.matmul(
                                            ps[:, fs:fs + fl],
                                            w1t[:, kc, :],
                                            h1t_sb[:, kc, fs:fs + fl],
                                            start=(kc == 0),
                                            stop=(kc == KO - 1),
                                        )
                                nc.scalar.activation(
                                    ft_sb[:, ci, :], ps[:, :TQ], AF.Gelu,
                                    bias=bf1_sb[:, c:c + 1])
                            for tq in range(TQT):
                                ps = f2_ps.tile([P, D], F32, tag="pg")
                                for ci in range(DH):
                                    for fs, fl in _chunks(D, 512):
                                        nc.tensor.matmul(
                                            ps[:, fs:fs + fl],
                                            ft_sb[:, ci,
                                                  tq * P:(tq + 1) * P],
                                            wf2_sb[:, ci, fs:fs + fl],
                                            start=(ci == 0),
                                            stop=(ci == DH - 1),
                                        )
                                if dh < NDH - 1:
                                    # accumulate partial FFN2 into h1 (h1 is
                                    # only needed again for the residual)
                                    nc.vector.tensor_add(
                                        h1_sb[:, tq, :], ps[:],
                                        h1_sb[:, tq, :])
                                else:
                                    resid = ln2_pool.tile([P, D], F32,
                                                          tag="r2")
                                    nc.vector.tensor_add(
                                        resid[:], ps[:], h1_sb[:, tq, :])
                                    nc.vector.tensor_add(
                                        resid[:], resid[:], bf2_b[:])
                                    st6 = ln2_pool.tile([P, 2, 6], F32,
                                                        tag="st6b")
                                    nc.vector.bn_stats(
                                        st6[:, 0, :], resid[:, 0:512])
                                    nc.vector.bn_stats(
                                        st6[:, 1, :], resid[:, 512:1024])
                                    mv = ln2_pool.tile([P, 2], F32,
                                                       tag="mvb")
                                    nc.vector.bn_aggr(mv[:], st6[:])
                                    rstd = ln2_pool.tile([P, 1], F32,
                                                         tag="rstdb")
                                    nc.scalar.activation(
                                        rstd[:], mv[:, 1:2], AF.Sqrt,
                                        bias=eps_sb[:, 0:1])
                                    nc.vector.reciprocal(rstd[:], rstd[:])
                                    out_t = ln2_pool.tile([P, D], F32,
                                                          tag="outt")
                                    nc.vector.tensor_scalar(
                                        out_t[:], resid[:], mv[:, 0:1],
                                        rstd[:],
                                        mybir.AluOpType.subtract,
                                        mybir.AluOpType.mult)
                                    nc.vector.tensor_mul(
                                        out_t[:], out_t[:], g2_b[:])
                                    nc.vector.tensor_add(
                                        out_t[:], out_t[:], b2_b[:])
                                    nc.sync.dma_start(
                                        y[tq * P:(tq + 1) * P, :], out_t[:])

    return nc


# ---------------------------------------------------------------------------
def _featmajor(a):
    """[T, D] fp32 -> [128, D//128, T] bf16 (feature-major chunked)."""
    t = a.shape[0]
    return np.ascontiguousarray(
        a.T.reshape(KO, P, t).transpose(1, 0, 2)).astype(BF)


def make_in_maps(x1, x2, Wq, bq, Wk, bk, Wv, bv, Wo, bo, g1, b1, g2, b2,
                 Wf1, bf1, Wf2, bf2, n_cores=8, TQ=1024):
    """Host-side prep: slice/cast/re-layout all inputs for each core."""
    B = x1.shape[0]
    per_batch = n_cores // B
    wq_l = np.ascontiguousarray(
        Wq.reshape(KO, P, D).transpose(1, 0, 2)).astype(BF)
    wk_l = np.ascontiguousarray(
        Wk.reshape(KO, P, D).transpose(1, 0, 2)).astype(BF)
    wv_l = np.ascontiguousarray(
        Wv.reshape(KO, P, D).transpose(1, 0, 2)).astype(BF)
    wo_l = np.ascontiguousarray(
        Wo.reshape(KO, P, D).transpose(1, 0, 2)).astype(BF)
    wf1_l = np.ascontiguousarray(
        Wf1.reshape(KO, P, DFFC, P).transpose(2, 1, 0, 3)).astype(BF)
    wf2_l = np.ascontiguousarray(Wf2.reshape(DFFC, P, D)).astype(BF)
    shared = {
        "wq": wq_l, "wk": wk_l, "wv": wv_l, "wo": wo_l,
        "wf1": wf1_l, "wf2": wf2_l,
        "bq": np.ascontiguousarray(
            np.asarray(bq, np.float32).reshape(KO, P).T),
        "bk": np.ascontiguousarray(
            np.asarray(bk, np.float32).reshape(KO, P).T),
        "bf1": np.ascontiguousarray(
            np.asarray(bf1, np.float32).reshape(DFFC, P).T),
        "bvr": np.asarray(bv, np.float32).reshape(1, D),
        "bf2r": np.asarray(bf2, np.float32).reshape(1, D),
        "g1r": np.asarray(g1, np.float32).reshape(1, D),
        "b1r": np.asarray(b1, np.float32).reshape(1, D),
        "g2r": np.asarray(g2, np.float32).reshape(1, D),
        "b2r": np.asarray(b2, np.float32).reshape(1, D),
    }
    x2t_b = [_featmajor(np.asarray(x2[b], np.float32)) for b in range(B)]
    in_maps = []
    for c in range(n_cores):
        b, half = c // per_batch, c % per_batch
        x1s = np.asarray(x1[b, half * TQ:(half + 1) * TQ, :], np.float32)
        in_maps.append(dict(
            shared,
            x1t=_featmajor(x1s),
            x1b=np.ascontiguousarray(x1s + np.asarray(bo, np.float32)),
            x2t=x2t_b[b],
        ))
    return in_maps


def kernel(x1, x2, Wq, bq, Wk, bk, Wv, bv, Wo, bo, g1, b1, g2, b2,
           Wf1, bf1, Wf2, bf2):
    x1 = np.asarray(x1, np.float32)
    x2 = np.asarray(x2, np.float32)
    B, S, _ = x1.shape
    TQ = (B * S) // 8
    in_maps = make_in_maps(x1, x2, Wq, bq, Wk, bk, Wv, bv, Wo, bo,
                           g1, b1, g2, b2, Wf1, bf1, Wf2, bf2, TQ=TQ)
    key = (TQ, S)
    if key not in _nc_cache:
        nc = build_nc(TQ=TQ, TK=S)
        # walrus-compat pass must run after (and only after) CoreSim use
        _split_multi_waits(nc)
        _nc_cache[key] = nc
    nc = _nc_cache[key]
    res = bass_utils.run_bass_kernel_spmd(nc, in_maps, core_ids=list(range(8)))
    ys = [r["y"] for r in res.results]
    return np.concatenate(ys, axis=0).reshape(B, S, D)
